# revision 38
# baseline (speedup 1.0000x reference)
"""GAT layer (B=8, N=2048, F=64) on 8 trn2 NeuronCores.

Strategy: data-parallel over batch B — one graph per core, adj replicated.

Math: with e = leaky_relu(e1_i + e2_j), exp(e - 0.2*e1_i) (row factor
cancels in softmax) = A2_j * max(G_i, r_j) where G = exp(0.8*e1),
A2 = exp(e2), r = exp(-0.8*e2). A2 folds into the matmul weights on the
host (whA = [Wh*A2 | A2]; row 64 yields softmax denominators), so the
device computes q_ij = max(G_i, r_j) * adj_ji and accumulates
outT[65, i] += whA_t^T @ q_t over 16 j-tiles. Divide + elu epilogue is
O(N*F) and runs on the host.

The N^2 elementwise stage runs entirely on DVE, whose fast modes need
all-2-byte SBUF operands (hence adj in fp16):
    s_t = (G max r_t)   tensor_scalar, 4x mode, ~0.75us/tile
    q_t = s_t * adj_t   tensor_tensor, 2x mode, ~1.2us/tile
Measured alternatives all lose: the fused scalar_tensor_tensor runs 1x
(~2.35us/tile), GpSimd/Pool tensor ops poison DVE's fast modes (2.5-6x
degradation while Pool runs), and gpsimd accumulating-DMA masking is
issue-bound and degrades every co-running engine.

DMA scheduling is the other measured bottleneck — bandwidth is shared
round-robin across in-flight transfers:
  - G (host-replicated; stride-0 broadcast DMAs are ~5x slower) issues
    FIRST as 2 chunks on the 2 hwdge queues so the score stream starts
    ~10.5us instead of ~19us.
  - adj arrives as staggered groups [1,1,2,4,4,4] so early tiles land
    at the mask cadence while keeping few transfers in flight during
    the G fill.
  - Junk matmuls keep PE busy through the fill so its clock ramps
    (2.4GHz needs ~3us continuous busy).
  - The 16-tile accumulation runs as two 4-bank PSUM phases; phase-A
    copies (split ACT/DVE) + halved output DMAs overlap phase B; the
    host adds the halves.
"""

import sys

import numpy as np

for _p in ("/opt/trn_rl_repo",):
    if _p not in sys.path:
        sys.path.insert(0, _p)

from contextlib import ExitStack

import concourse.bass as bass
import concourse.tile as tile
from concourse import bacc, mybir
from concourse.bass_utils import run_bass_kernel_spmd

B, N, F = 8, 2048, 64
P = 128
T = N // P  # 16 j-tiles
NB = N // 512  # 4 psum banks of moving-free 512
ADJ_GROUPS = [(0,), (1,), (2, 3), (4, 5, 6, 7), (8, 9, 10, 11), (12, 13, 14, 15)]
PREFETCH = 3  # score PAIRS emitted ahead of the mask loop
NWARM = 8  # PE clock-ramp matmuls during the DMA fill

_CACHE = {}


def _build_program():
    if "nc" in _CACHE:
        return _CACHE["nc"]
    dt = mybir.dt
    nc = bacc.Bacc("TRN2", target_bir_lowering=False, debug=False)

    adjd = nc.dram_tensor("adjd", [P, T * N], dt.float16, kind="ExternalInput").ap()
    g = nc.dram_tensor("g", [P, N], dt.float16, kind="ExternalInput").ap()
    rsc = nc.dram_tensor("rsc", [P, T], dt.float32, kind="ExternalInput").ap()
    wha = nc.dram_tensor("wha", [P, T * 65], dt.float16, kind="ExternalInput").ap()
    outA = nc.dram_tensor("outA", [65, N], dt.float16, kind="ExternalOutput").ap()
    outB = nc.dram_tensor("outB", [65, N], dt.float16, kind="ExternalOutput").ap()

    with tile.TileContext(nc) as tc, ExitStack() as ctx:
        singles = ctx.enter_context(tc.tile_pool(name="singles", bufs=1))
        spool = ctx.enter_context(tc.tile_pool(name="spool", bufs=PREFETCH + 1))
        qpool = ctx.enter_context(tc.tile_pool(name="qpool", bufs=3))
        accp = ctx.enter_context(tc.tile_pool(name="accp", bufs=1, space="PSUM"))

        # G first: its 2 chunks see the least bandwidth sharing.
        g_sb = singles.tile([P, N], dt.float16)
        nc.sync.dma_start(out=g_sb[:, 0:1024], in_=g[:, 0:1024])
        nc.scalar.dma_start(out=g_sb[:, 1024:2048], in_=g[:, 1024:2048])
        rsc_sb = singles.tile([P, T], dt.float32)
        nc.scalar.dma_start(out=rsc_sb[:], in_=rsc)

        # adj issues ride the Scalar queue BEHIND g/rsc, so G's transfers see
        # a near-empty fabric and the score stream starts earlier; wha (first
        # needed by the t=0 matmul, ~5us later) slots in after the two
        # single-tile adj groups.
        adj_sb = singles.tile([P, T * N], dt.float16, name="adj")
        wha_sb = singles.tile([P, T * 65], dt.float16)

        def adj_dma(grp):
            lo, hi = grp[0], grp[-1] + 1
            nc.scalar.dma_start(
                out=adj_sb[:, lo * N : hi * N], in_=adjd[:, lo * N : hi * N]
            )

        adj_dma(ADJ_GROUPS[0])
        adj_dma(ADJ_GROUPS[1])
        nc.scalar.dma_start(out=wha_sb[:], in_=wha)
        for grp in ADJ_GROUPS[2:]:
            adj_dma(grp)

        def make_spair(pr):
            sp = spool.tile([P, 2 * N], dt.float16, name="s")
            for k in range(2):
                t = 2 * pr + k
                nc.vector.tensor_scalar_max(
                    sp[:, k * N : (k + 1) * N], g_sb[:], rsc_sb[:, t : t + 1]
                )
            return sp

        spairs = {pr: make_spair(pr) for pr in range(PREFETCH)}

        accs = {}
        for ph in range(2):
            for n in range(NB):
                accs[ph, n] = accp.tile(
                    [65, 512], dt.float32, tag=f"acc{ph}_{n}", name=f"acc{ph}_{n}"
                )

        # PE clock-ramp during the fill, reading only g_sb (available early).
        for w in range(NWARM):
            nc.tensor.matmul(
                out=accs[1, w % NB][:],
                lhsT=g_sb[:, 0:65],
                rhs=g_sb[:, 0:512],
                start=True,
                stop=True,
            )

        osbA = singles.tile([65, N], dt.float16, name="osbA")
        osbB = singles.tile([65, N], dt.float16, name="osbB")

        for pr in range(T // 2):
            sp = spairs.pop(pr) if pr in spairs else make_spair(pr)
            # One 2x-mode mask over the whole pair: amortizes the per-op
            # init/dispatch overhead (~120ns) over 2 tiles.
            qp = qpool.tile([P, 2 * N], dt.float16)
            nc.vector.tensor_tensor(
                qp[:], sp[:], adj_sb[:, 2 * pr * N : (2 * pr + 2) * N],
                mybir.AluOpType.mult,
            )
            for k in range(2):
                t = 2 * pr + k
                ph, first, last = t // 8, t % 8 == 0, t % 8 == 7
                for n in range(NB):
                    nc.tensor.matmul(
                        out=accs[ph, n][:],
                        lhsT=wha_sb[:, t * 65 : (t + 1) * 65],
                        rhs=qp[:, k * N + n * 512 : k * N + (n + 1) * 512],
                        start=first,
                        stop=last,
                    )
                if last:
                    # Copy banks 0/1 (ACT + DVE in parallel), ship that half
                    # while banks 2/3 copy, then ship the other half.
                    osb, outd = (osbA, outA) if ph == 0 else (osbB, outB)
                    nc.scalar.copy(osb[:, 0:512], accs[ph, 0][:])
                    nc.vector.tensor_copy(osb[:, 512:1024], accs[ph, 1][:])
                    nc.sync.dma_start(
                        out=outd[:, 0 : N // 2], in_=osb[:, 0 : N // 2]
                    )
                    nc.scalar.copy(osb[:, 1024:1536], accs[ph, 2][:])
                    nc.vector.tensor_copy(osb[:, 1536:2048], accs[ph, 3][:])
                    nc.scalar.dma_start(out=outd[:, N // 2 :], in_=osb[:, N // 2 :])

    nc.compile()
    _CACHE["nc"] = nc
    return nc


def _prep_inputs(h, adj, W, a):
    h = np.asarray(h, np.float32)
    adj = np.asarray(adj, np.float32)
    W = np.asarray(W, np.float32)
    a = np.asarray(a, np.float32)

    # adj^T tiles side by side along free dim: adjd[p, t*N + i] = adjT[t*128+p, i]
    adjd = np.ascontiguousarray(
        adj.T.reshape(T, P, N).transpose(1, 0, 2).reshape(P, T * N)
    ).astype(np.float16)

    Wh = np.einsum("bnf,of->bno", h, W)  # [B, N, F]
    e1 = Wh @ a[:F]  # [B, N]
    e2 = Wh @ a[F:]  # [B, N]
    A2 = np.exp(e2)
    G = np.exp(0.8 * e1).astype(np.float16)  # [B, N]
    r = np.exp(-0.8 * e2).astype(np.float32)  # [B, N]
    whA = np.concatenate([Wh * A2[..., None], A2[..., None]], axis=2)  # [B, N, 65]
    whA = np.ascontiguousarray(
        whA.reshape(B, T, P, 65).transpose(0, 2, 1, 3)
    ).reshape(B, P, T * 65)

    in_maps = []
    for b in range(B):
        in_maps.append(
            {
                "adjd": adjd,
                "g": np.ascontiguousarray(np.broadcast_to(G[b], (P, N))),
                "rsc": np.ascontiguousarray(r[b].reshape(T, P).T),
                "wha": whA[b].astype(np.float16),
            }
        )
    return in_maps


def kernel(h, adj, W, a, _trace=False):
    nc = _build_program()
    in_maps = _prep_inputs(h, adj, W, a)
    res = run_bass_kernel_spmd(nc, in_maps, list(range(B)), trace=_trace)
    outs = np.empty((B, N, F), np.float32)
    for b in range(B):
        outT = np.asarray(res.results[b]["outA"], np.float32) + np.asarray(
            res.results[b]["outB"], np.float32
        )
        hp = outT[:F].T / outT[F][:, None]
        outs[b] = np.where(hp > 0, hp, np.expm1(hp))
    if _trace:
        kernel.last_results = res
    return outs


# revision 41
# speedup vs baseline: 1.0116x; 1.0116x over previous
"""GAT layer (B=8, N=2048, F=64) on 8 trn2 NeuronCores.

Strategy: data-parallel over batch B — one graph per core, adj replicated.

Math: with e = leaky_relu(e1_i + e2_j), exp(e - 0.2*e1_i) (row factor
cancels in softmax) = A2_j * max(G_i, r_j) where G = exp(0.8*e1),
A2 = exp(e2), r = exp(-0.8*e2). A2 folds into the matmul weights on the
host (whA = [Wh*A2 | A2]; row 64 yields softmax denominators), so the
device computes q_ij = max(G_i, r_j) * adj_ji and accumulates
outT[65, i] += whA_t^T @ q_t over 16 j-tiles. Divide + elu epilogue is
O(N*F) and runs on the host.

The N^2 elementwise stage runs entirely on DVE, whose fast modes need
all-2-byte SBUF operands (hence adj in fp16):
    s_t = (G max r_t)   tensor_scalar, 4x mode, ~0.75us/tile
    q_t = s_t * adj_t   tensor_tensor, 2x mode, ~1.2us/tile
Measured alternatives all lose: the fused scalar_tensor_tensor runs 1x
(~2.35us/tile), GpSimd/Pool tensor ops poison DVE's fast modes (2.5-6x
degradation while Pool runs), and gpsimd accumulating-DMA masking is
issue-bound and degrades every co-running engine.

DMA scheduling is the other measured bottleneck — bandwidth is shared
round-robin across in-flight transfers:
  - G (host-replicated; stride-0 broadcast DMAs are ~5x slower) issues
    FIRST as 2 chunks on the 2 hwdge queues so the score stream starts
    ~10.5us instead of ~19us.
  - adj arrives as staggered groups [1,1,2,4,4,4] so early tiles land
    at the mask cadence while keeping few transfers in flight during
    the G fill.
  - Junk matmuls keep PE busy through the fill so its clock ramps
    (2.4GHz needs ~3us continuous busy).
  - The 16-tile accumulation runs as two 4-bank PSUM phases; phase-A
    copies (split ACT/DVE) + halved output DMAs overlap phase B; the
    host adds the halves.
"""

import sys

import numpy as np

for _p in ("/opt/trn_rl_repo",):
    if _p not in sys.path:
        sys.path.insert(0, _p)

from contextlib import ExitStack

import concourse.bass as bass
import concourse.tile as tile
from concourse import bacc, mybir
from concourse.bass_utils import run_bass_kernel_spmd

B, N, F = 8, 2048, 64
P = 128
T = N // P  # 16 j-tiles
NB = N // 512  # 4 psum banks of moving-free 512
ADJ_GROUPS = [(0,), (1,), (2, 3), (4, 5, 6, 7), (8, 9, 10, 11), (12, 13, 14, 15)]
PREFETCH = 3  # score PAIRS emitted ahead of the mask loop
NWARM = 8  # PE clock-ramp matmuls during the DMA fill

_CACHE = {}


def _build_program():
    if "nc" in _CACHE:
        return _CACHE["nc"]
    dt = mybir.dt
    nc = bacc.Bacc("TRN2", target_bir_lowering=False, debug=False)

    adjd = nc.dram_tensor("adjd", [P, T * N], dt.float16, kind="ExternalInput").ap()
    g = nc.dram_tensor("g", [P, N], dt.float16, kind="ExternalInput").ap()
    rsc = nc.dram_tensor("rsc", [P, T], dt.float32, kind="ExternalInput").ap()
    wha = nc.dram_tensor("wha", [P, T * 65], dt.float16, kind="ExternalInput").ap()
    outA = nc.dram_tensor("outA", [65, N], dt.float16, kind="ExternalOutput").ap()
    outB = nc.dram_tensor("outB", [65, N], dt.float16, kind="ExternalOutput").ap()

    with tile.TileContext(nc) as tc, ExitStack() as ctx:
        singles = ctx.enter_context(tc.tile_pool(name="singles", bufs=1))
        spool = ctx.enter_context(tc.tile_pool(name="spool", bufs=PREFETCH + 1))
        qpool = ctx.enter_context(tc.tile_pool(name="qpool", bufs=3))
        accp = ctx.enter_context(tc.tile_pool(name="accp", bufs=1, space="PSUM"))

        # G first: its 2 chunks see the least bandwidth sharing.
        g_sb = singles.tile([P, N], dt.float16)
        nc.sync.dma_start(out=g_sb[:, 0:1024], in_=g[:, 0:1024])
        nc.scalar.dma_start(out=g_sb[:, 1024:2048], in_=g[:, 1024:2048])
        rsc_sb = singles.tile([P, T], dt.float32)
        nc.scalar.dma_start(out=rsc_sb[:], in_=rsc)
        wha_sb = singles.tile([P, T * 65], dt.float16)
        nc.scalar.dma_start(out=wha_sb[:], in_=wha)

        # adj issues ride the Scalar queue BEHIND g/rsc/wha, so G's transfers
        # see a near-empty fabric and the score stream starts ~3us earlier.
        adj_sb = singles.tile([P, T * N], dt.float16, name="adj")
        for grp in ADJ_GROUPS:
            lo, hi = grp[0], grp[-1] + 1
            nc.scalar.dma_start(
                out=adj_sb[:, lo * N : hi * N], in_=adjd[:, lo * N : hi * N]
            )

        # Mask granularity: singles at the stream's ends (earlier PE start,
        # shorter PE tail), overhead-amortizing pairs in the middle.
        MGROUPS = [(0,), (1,), (2, 3), (4, 5), (6, 7), (8, 9), (10, 11),
                   (12, 13), (14,), (15,)]

        def make_sgrp(gi):
            grp = MGROUPS[gi]
            sp = spool.tile([P, len(grp) * N], dt.float16, name="s")
            for k, t in enumerate(grp):
                nc.vector.tensor_scalar_max(
                    sp[:, k * N : (k + 1) * N], g_sb[:], rsc_sb[:, t : t + 1]
                )
            return sp

        sgrps = {gi: make_sgrp(gi) for gi in range(PREFETCH)}

        accs = {}
        for ph in range(2):
            for n in range(NB):
                accs[ph, n] = accp.tile(
                    [65, 512], dt.float32, tag=f"acc{ph}_{n}", name=f"acc{ph}_{n}"
                )

        # PE clock-ramp during the fill, reading only g_sb (available early).
        for w in range(NWARM):
            nc.tensor.matmul(
                out=accs[1, w % NB][:],
                lhsT=g_sb[:, 0:65],
                rhs=g_sb[:, 0:512],
                start=True,
                stop=True,
            )

        osbA = singles.tile([65, N], dt.float16, name="osbA")
        osbB = singles.tile([65, N], dt.float16, name="osbB")

        for gi in range(len(MGROUPS)):
            grp = MGROUPS[gi]
            sp = sgrps.pop(gi) if gi in sgrps else make_sgrp(gi)
            lo = grp[0]
            qp = qpool.tile([P, len(grp) * N], dt.float16, name="q")
            nc.vector.tensor_tensor(
                qp[:], sp[:], adj_sb[:, lo * N : (lo + len(grp)) * N],
                mybir.AluOpType.mult,
            )
            for k, t in enumerate(grp):
                ph, first, last = t // 8, t % 8 == 0, t % 8 == 7
                for n in range(NB):
                    nc.tensor.matmul(
                        out=accs[ph, n][:],
                        lhsT=wha_sb[:, t * 65 : (t + 1) * 65],
                        rhs=qp[:, k * N + n * 512 : k * N + (n + 1) * 512],
                        start=first,
                        stop=last,
                    )
                if last:
                    # Copy banks 0/1 (ACT + DVE in parallel), ship that half
                    # while banks 2/3 copy, then ship the other half.
                    osb, outd = (osbA, outA) if ph == 0 else (osbB, outB)
                    nc.scalar.copy(osb[:, 0:512], accs[ph, 0][:])
                    nc.vector.tensor_copy(osb[:, 512:1024], accs[ph, 1][:])
                    nc.sync.dma_start(
                        out=outd[:, 0 : N // 2], in_=osb[:, 0 : N // 2]
                    )
                    nc.scalar.copy(osb[:, 1024:1536], accs[ph, 2][:])
                    nc.vector.tensor_copy(osb[:, 1536:2048], accs[ph, 3][:])
                    nc.scalar.dma_start(out=outd[:, N // 2 :], in_=osb[:, N // 2 :])

    nc.compile()
    _CACHE["nc"] = nc
    return nc


def _prep_inputs(h, adj, W, a):
    h = np.asarray(h, np.float32)
    adj = np.asarray(adj, np.float32)
    W = np.asarray(W, np.float32)
    a = np.asarray(a, np.float32)

    # adj^T tiles side by side along free dim: adjd[p, t*N + i] = adjT[t*128+p, i]
    adjd = np.ascontiguousarray(
        adj.T.reshape(T, P, N).transpose(1, 0, 2).reshape(P, T * N)
    ).astype(np.float16)

    Wh = np.einsum("bnf,of->bno", h, W)  # [B, N, F]
    e1 = Wh @ a[:F]  # [B, N]
    e2 = Wh @ a[F:]  # [B, N]
    A2 = np.exp(e2)
    G = np.exp(0.8 * e1).astype(np.float16)  # [B, N]
    r = np.exp(-0.8 * e2).astype(np.float32)  # [B, N]
    whA = np.concatenate([Wh * A2[..., None], A2[..., None]], axis=2)  # [B, N, 65]
    whA = np.ascontiguousarray(
        whA.reshape(B, T, P, 65).transpose(0, 2, 1, 3)
    ).reshape(B, P, T * 65)

    in_maps = []
    for b in range(B):
        in_maps.append(
            {
                "adjd": adjd,
                "g": np.ascontiguousarray(np.broadcast_to(G[b], (P, N))),
                "rsc": np.ascontiguousarray(r[b].reshape(T, P).T),
                "wha": whA[b].astype(np.float16),
            }
        )
    return in_maps


def kernel(h, adj, W, a, _trace=False):
    nc = _build_program()
    in_maps = _prep_inputs(h, adj, W, a)
    res = run_bass_kernel_spmd(nc, in_maps, list(range(B)), trace=_trace)
    outs = np.empty((B, N, F), np.float32)
    for b in range(B):
        outT = np.asarray(res.results[b]["outA"], np.float32) + np.asarray(
            res.results[b]["outB"], np.float32
        )
        hp = outT[:F].T / outT[F][:, None]
        outs[b] = np.where(hp > 0, hp, np.expm1(hp))
    if _trace:
        kernel.last_results = res
    return outs


# revision 44
# speedup vs baseline: 1.0769x; 1.0645x over previous
"""GAT layer (B=8, N=2048, F=64) on 8 trn2 NeuronCores.

Strategy: data-parallel over batch B — one graph per core, adj replicated.

Math: with e = leaky_relu(e1_i + e2_j), exp(e - 0.2*e1_i) (row factor
cancels in softmax) = A2_j * max(G_i, r_j) where G = exp(0.8*e1),
A2 = exp(e2), r = exp(-0.8*e2). A2 folds into the matmul weights on the
host (whA = [Wh*A2 | A2]; row 64 yields softmax denominators), so the
device computes q_ij = max(G_i, r_j) * adj_ji and accumulates
outT[65, i] += whA_t^T @ q_t over 16 j-tiles. Divide + elu epilogue is
O(N*F) and runs on the host.

The N^2 elementwise stage runs entirely on DVE, whose fast modes need
all-2-byte SBUF operands (hence adj in fp16):
    s_t = (G max r_t)   tensor_scalar, 4x mode, ~0.75us/tile
    q_t = s_t * adj_t   tensor_tensor, 2x mode, ~1.2us/tile
Measured alternatives all lose: the fused scalar_tensor_tensor runs 1x
(~2.35us/tile), GpSimd/Pool tensor ops poison DVE's fast modes (2.5-6x
degradation while Pool runs), and gpsimd accumulating-DMA masking is
issue-bound and degrades every co-running engine.

DMA scheduling is the other measured bottleneck — bandwidth is shared
round-robin across in-flight transfers:
  - G (host-replicated; stride-0 broadcast DMAs are ~5x slower) issues
    FIRST as 2 chunks on the 2 hwdge queues so the score stream starts
    ~10.5us instead of ~19us.
  - adj arrives as staggered groups [1,1,2,4,4,4] so early tiles land
    at the mask cadence while keeping few transfers in flight during
    the G fill.
  - Junk matmuls keep PE busy through the fill so its clock ramps
    (2.4GHz needs ~3us continuous busy).
  - The 16-tile accumulation runs as two 4-bank PSUM phases; phase-A
    copies (split ACT/DVE) + halved output DMAs overlap phase B; the
    host adds the halves.
"""

import sys

import numpy as np

for _p in ("/opt/trn_rl_repo",):
    if _p not in sys.path:
        sys.path.insert(0, _p)

from contextlib import ExitStack

import concourse.bass as bass
import concourse.tile as tile
from concourse import bacc, mybir
from concourse.bass_utils import run_bass_kernel_spmd

B, N, F = 8, 2048, 64
P = 128
T = N // P  # 16 j-tiles
NB = N // 512  # 4 psum banks of moving-free 512
ADJ_GROUPS = [(0,), (1,), (2, 3), (4, 5, 6, 7), (8, 9, 10, 11), (12, 13, 14, 15)]
PREFETCH = 6  # score groups emitted ahead of the mask loop
NWARM = 8  # PE clock-ramp matmuls during the DMA fill

_CACHE = {}


def _build_program():
    if "nc" in _CACHE:
        return _CACHE["nc"]
    dt = mybir.dt
    nc = bacc.Bacc("TRN2", target_bir_lowering=False, debug=False)

    adjd = nc.dram_tensor("adjd", [P, T * N], dt.float16, kind="ExternalInput").ap()
    g = nc.dram_tensor("g", [P, N], dt.float16, kind="ExternalInput").ap()
    rsc = nc.dram_tensor("rsc", [P, T], dt.float32, kind="ExternalInput").ap()
    wha = nc.dram_tensor("wha", [P, T * 65], dt.float16, kind="ExternalInput").ap()
    outA = nc.dram_tensor("outA", [65, N], dt.float16, kind="ExternalOutput").ap()
    outB = nc.dram_tensor("outB", [65, N], dt.float16, kind="ExternalOutput").ap()

    with tile.TileContext(nc) as tc, ExitStack() as ctx:
        singles = ctx.enter_context(tc.tile_pool(name="singles", bufs=1))
        spool = ctx.enter_context(tc.tile_pool(name="spool", bufs=PREFETCH + 2))
        qpool = ctx.enter_context(tc.tile_pool(name="qpool", bufs=6))
        accp = ctx.enter_context(tc.tile_pool(name="accp", bufs=1, space="PSUM"))

        # G first: its 2 chunks see the least bandwidth sharing.
        g_sb = singles.tile([P, N], dt.float16)
        nc.sync.dma_start(out=g_sb[:, 0:1024], in_=g[:, 0:1024])
        nc.scalar.dma_start(out=g_sb[:, 1024:2048], in_=g[:, 1024:2048])
        rsc_sb = singles.tile([P, T], dt.float32)
        nc.scalar.dma_start(out=rsc_sb[:], in_=rsc)
        wha_sb = singles.tile([P, T * 65], dt.float16)
        nc.scalar.dma_start(out=wha_sb[:], in_=wha)

        # adj issues ride the Scalar queue BEHIND g/rsc/wha, so G's transfers
        # see a near-empty fabric and the score stream starts ~3us earlier.
        adj_sb = singles.tile([P, T * N], dt.float16, name="adj")
        for grp in ADJ_GROUPS:
            lo, hi = grp[0], grp[-1] + 1
            nc.scalar.dma_start(
                out=adj_sb[:, lo * N : hi * N], in_=adjd[:, lo * N : hi * N]
            )

        # Mask granularity: single tiles measured most robust (pair-merged
        # masks save ~70ns/tile of op overhead but coarsen the pipeline and
        # lost as often as they won across runs).
        MGROUPS = [(t,) for t in range(T)]

        def make_sgrp(gi):
            grp = MGROUPS[gi]
            sp = spool.tile([P, len(grp) * N], dt.float16, name="s")
            for k, t in enumerate(grp):
                nc.vector.tensor_scalar_max(
                    sp[:, k * N : (k + 1) * N], g_sb[:], rsc_sb[:, t : t + 1]
                )
            return sp

        sgrps = {gi: make_sgrp(gi) for gi in range(PREFETCH)}

        accs = {}
        for ph in range(2):
            for n in range(NB):
                accs[ph, n] = accp.tile(
                    [65, 512], dt.float32, tag=f"acc{ph}_{n}", name=f"acc{ph}_{n}"
                )

        # PE clock-ramp during the fill, reading only g_sb (available early).
        for w in range(NWARM):
            nc.tensor.matmul(
                out=accs[1, w % NB][:],
                lhsT=g_sb[:, 0:65],
                rhs=g_sb[:, 0:512],
                start=True,
                stop=True,
            )

        osbA = singles.tile([65, N], dt.float16, name="osbA")
        osbB = singles.tile([65, N], dt.float16, name="osbB")

        for gi in range(len(MGROUPS)):
            grp = MGROUPS[gi]
            sp = sgrps.pop(gi) if gi in sgrps else make_sgrp(gi)
            lo = grp[0]
            qp = qpool.tile([P, len(grp) * N], dt.float16, name="q")
            nc.vector.tensor_tensor(
                qp[:], sp[:], adj_sb[:, lo * N : (lo + len(grp)) * N],
                mybir.AluOpType.mult,
            )
            for k, t in enumerate(grp):
                ph, first, last = t // 8, t % 8 == 0, t % 8 == 7
                for n in range(NB):
                    nc.tensor.matmul(
                        out=accs[ph, n][:],
                        lhsT=wha_sb[:, t * 65 : (t + 1) * 65],
                        rhs=qp[:, k * N + n * 512 : k * N + (n + 1) * 512],
                        start=first,
                        stop=last,
                    )
                if last:
                    # Copy banks 0/1 (ACT + DVE in parallel), ship that half
                    # while banks 2/3 copy, then ship the other half.
                    osb, outd = (osbA, outA) if ph == 0 else (osbB, outB)
                    nc.scalar.copy(osb[:, 0:512], accs[ph, 0][:])
                    nc.vector.tensor_copy(osb[:, 512:1024], accs[ph, 1][:])
                    nc.sync.dma_start(
                        out=outd[:, 0 : N // 2], in_=osb[:, 0 : N // 2]
                    )
                    nc.scalar.copy(osb[:, 1024:1536], accs[ph, 2][:])
                    nc.vector.tensor_copy(osb[:, 1536:2048], accs[ph, 3][:])
                    nc.scalar.dma_start(out=outd[:, N // 2 :], in_=osb[:, N // 2 :])

    nc.compile()
    _CACHE["nc"] = nc
    return nc


def _prep_inputs(h, adj, W, a):
    h = np.asarray(h, np.float32)
    adj = np.asarray(adj, np.float32)
    W = np.asarray(W, np.float32)
    a = np.asarray(a, np.float32)

    # adj^T tiles side by side along free dim: adjd[p, t*N + i] = adjT[t*128+p, i]
    adjd = np.ascontiguousarray(
        adj.T.reshape(T, P, N).transpose(1, 0, 2).reshape(P, T * N)
    ).astype(np.float16)

    Wh = np.einsum("bnf,of->bno", h, W)  # [B, N, F]
    e1 = Wh @ a[:F]  # [B, N]
    e2 = Wh @ a[F:]  # [B, N]
    A2 = np.exp(e2)
    G = np.exp(0.8 * e1).astype(np.float16)  # [B, N]
    r = np.exp(-0.8 * e2).astype(np.float32)  # [B, N]
    whA = np.concatenate([Wh * A2[..., None], A2[..., None]], axis=2)  # [B, N, 65]
    whA = np.ascontiguousarray(
        whA.reshape(B, T, P, 65).transpose(0, 2, 1, 3)
    ).reshape(B, P, T * 65)

    in_maps = []
    for b in range(B):
        in_maps.append(
            {
                "adjd": adjd,
                "g": np.ascontiguousarray(np.broadcast_to(G[b], (P, N))),
                "rsc": np.ascontiguousarray(r[b].reshape(T, P).T),
                "wha": whA[b].astype(np.float16),
            }
        )
    return in_maps


def kernel(h, adj, W, a, _trace=False):
    nc = _build_program()
    in_maps = _prep_inputs(h, adj, W, a)
    res = run_bass_kernel_spmd(nc, in_maps, list(range(B)), trace=_trace)
    outs = np.empty((B, N, F), np.float32)
    for b in range(B):
        outT = np.asarray(res.results[b]["outA"], np.float32) + np.asarray(
            res.results[b]["outB"], np.float32
        )
        hp = outT[:F].T / outT[F][:, None]
        outs[b] = np.where(hp > 0, hp, np.expm1(hp))
    if _trace:
        kernel.last_results = res
    return outs
